# revision 8
# baseline (speedup 1.0000x reference)
"""GroupDRO kernel v2': bf16 value chain, fused-STT masked accumulation.

Changes vs v1 (all DVE-cost motivated, from InstructionCostModel):
  - E planes, se, L, labf, dark, v, t, u in bf16 (TT ops get the 2x packed
    16-bit DVE mode; STT stays 1x but is the only accumulate that works here)
  - dark via E0*E1*E2 < e^1.2 (2 TT-2x + TSP-4x) instead of 2 f32 STT ops
  - v = L - x_c stays mixed bf16/f32 (tested OK on HW)
Accumulation: scalar_tensor_tensor accum_out (the only fused reduce that
compiles+runs on this walrus path: TSP-accum, tiny matmul, TTR all fail).
"""
import sys

for _p in ("/opt/trn_rl_repo", "/opt/pypackages"):
    if _p not in sys.path:
        sys.path.insert(0, _p)

import numpy as np

N = 16777216
NCORES = 8
S = N // NCORES
P = 128
FS = 1024
TILE = P * FS
NT = S // TILE
BAL_ACC_W = 0.3
E12 = 3.3201169227365472  # e^1.2
REPEAT = 1

_cache = {}


def _build_nc():
    import concourse.bacc as bacc
    import concourse.tile as tile
    from concourse import mybir

    f32 = mybir.dt.float32
    bf16 = mybir.dt.bfloat16
    i32 = mybir.dt.int32
    Exp = mybir.ActivationFunctionType.Exp
    Ln = mybir.ActivationFunctionType.Ln
    Alu = mybir.AluOpType

    nc = bacc.Bacc("TRN2", target_bir_lowering=False, debug=False)
    logits_d = nc.dram_tensor("logits", [S, 3], f32, kind="ExternalInput")
    labels_d = nc.dram_tensor("labels", [S], i32, kind="ExternalInput")
    out_d = nc.dram_tensor("out", [P, 6], f32, kind="ExternalOutput")

    lg = logits_d.ap().rearrange("(t p s) c -> t p (s c)", t=NT, p=P, s=FS)
    lb = labels_d.ap().rearrange("(t p s) -> t p s", t=NT, p=P, s=FS)

    with tile.TileContext(nc) as tc:
        with (
            tc.tile_pool(name="io", bufs=3) as iop,
            tc.tile_pool(name="work", bufs=2) as wp,
            tc.tile_pool(name="acc", bufs=1) as ap_,
        ):
            accbuf = ap_.tile([P, 6 * NT], f32)

            for t in [tt for _ in range(REPEAT) for tt in range(NT)]:
                raw = iop.tile([P, FS * 3], f32, tag="raw")
                lab = iop.tile([P, FS], i32, tag="lab")
                nc.sync.dma_start(raw[:], lg[t])
                nc.sync.dma_start(lab[:], lb[t])
                raw3 = raw[:].rearrange("p (s c) -> p s c", c=3)

                E = wp.tile([P, 3, FS], bf16, tag="E")
                for c in range(3):
                    nc.scalar.activation(E[:, c, :], raw3[:, :, c], Exp)
                se = wp.tile([P, FS], bf16, tag="se")
                nc.vector.tensor_add(se[:], E[:, 0, :], E[:, 1, :])
                nc.vector.tensor_add(se[:], se[:], E[:, 2, :])
                L = wp.tile([P, FS], bf16, tag="L")
                nc.scalar.activation(L[:], se[:], Ln)

                labf = wp.tile([P, FS], bf16, tag="labf")
                nc.vector.tensor_copy(labf[:], lab[:])

                # dark = (x0+x1+x2) < 1.2  <=>  E0*E1*E2 < e^1.2
                prod = wp.tile([P, FS], bf16, tag="prod")
                nc.vector.tensor_mul(prod[:], E[:, 0, :], E[:, 1, :])
                nc.vector.tensor_mul(prod[:], prod[:], E[:, 2, :])
                dark = wp.tile([P, FS], bf16, tag="dark")
                nc.vector.tensor_scalar(dark[:], prod[:], E12, None, Alu.is_lt)

                for c in range(3):
                    v = wp.tile([P, FS], bf16, tag=f"v{c}")
                    nc.vector.tensor_sub(v[:], L[:], raw3[:, :, c])
                    tcl = wp.tile([P, FS], bf16, tag=f"t{c}")
                    nc.vector.scalar_tensor_tensor(
                        tcl[:], labf[:], float(c), v[:], Alu.is_equal, Alu.mult,
                        accum_out=accbuf[:, (c * NT + t):(c * NT + t + 1)],
                    )
                    u = wp.tile([P, FS], bf16, tag=f"u{c}")
                    nc.vector.scalar_tensor_tensor(
                        u[:], tcl[:], 1.0, dark[:], Alu.mult, Alu.mult,
                        accum_out=accbuf[:, ((3 + c) * NT + t):((3 + c) * NT + t + 1)],
                    )

            acc3 = accbuf[:].rearrange("p (q t) -> p q t", q=6)
            acc6 = ap_.tile([P, 6], f32)
            nc.vector.reduce_sum(acc6[:], acc3, axis=mybir.AxisListType.X)
            nc.sync.dma_start(out_d[:], acc6[:])

    nc.compile()
    return nc


def _get_nc():
    if "nc" not in _cache:
        _cache["nc"] = _build_nc()
    return _cache["nc"]


def run_device(logits, labels, trace=False, **kw):
    from concourse.bass_utils import run_bass_kernel_spmd

    nc = _get_nc()
    logits = np.ascontiguousarray(logits, dtype=np.float32).reshape(-1, 3)
    labels = np.ascontiguousarray(labels, dtype=np.int32).reshape(-1)
    in_maps = [
        {
            "logits": logits[i * S:(i + 1) * S],
            "labels": labels[i * S:(i + 1) * S],
        }
        for i in range(NCORES)
    ]
    br = run_bass_kernel_spmd(nc, in_maps, list(range(NCORES)), trace=trace, **kw)
    partials = np.stack(
        [r["out"].astype(np.float64).sum(axis=0) for r in br.results]
    )
    return partials, br


def kernel(logits, labels, group_weights):
    partials, _ = run_device(logits, labels)
    agg = partials.sum(0)
    T, U = agg[:3], agg[3:]
    subgroup = np.concatenate([U, T - U])
    group = np.array([U.sum(), (T - U).sum()])
    gw = np.asarray(group_weights, dtype=np.float64)
    standard = T.sum() / N
    combined = (1.0 - BAL_ACC_W) * standard + BAL_ACC_W * float(group @ gw)
    return (
        np.float32(combined),
        group.astype(np.float32),
        subgroup.astype(np.float32),
    )
